# revision 26
# baseline (speedup 1.0000x reference)
"""Bidirectional GRU encoder (Keras GRUCell reset_after=True) on Trainium2.

Problem shapes (hardcoded): V=32000, E=512, U=1024, B=32, T=256.

Strategy
--------
The time recurrence is strictly sequential, and the per-step cost is dominated
by streaming U_r (1024x3072) through the PE — independent of batch size. So
batch sharding buys nothing for the recurrence; instead the two directions run
on different cores (SPMD: one program, per-core *data* selects the direction —
core 0 gets forward inputs, core 1 gets time-reversed inputs).

Everything on-chip lives in a "transposed" layout with the gate/hidden dim on
partitions:

  hT   [128, KT*B]   hT[p, 32k+b]    = h[b, 128k+p]         (KT=8 chunks of U)
  G    [128, GT*B]   G[p, 32j+b]     = (h @ U_r)[b, 128j+p] (GT=24 tiles of 3U)
  xwT  [128, GT*B]   same layout, precomputed x @ W per step

This keeps every elementwise gate op at full 128-partition width and the
updated hT is directly the next step's matmul operand (no transposes in the
recurrent loop).

The input projection xW = emb[x] @ W is computed on-device in blocks of
TBLK=16 steps (512 tokens), double buffered in SBUF, interleaved with the
recurrence inside one For_i loop — it rides in the PE's idle slots, so it is
almost free and never touches DRAM.

Matmuls run in bf16 with fp32 PSUM accumulation.
"""

import numpy as np

V, E, U, B, T = 32000, 512, 1024, 32, 256
G = 3 * U            # 3072 gate width (z|r|n)
KT = U // 128        # 8  k-chunks of the hidden dim
GT = G // 128        # 24 g-tiles of the gate dim
ET = E // 128        # 4  e-chunks of the embedding dim
TBLK = 16            # recurrence steps per xW block (512 tokens)
NBLK = T // TBLK     # 16 blocks
TOKB = TBLK * B      # 512 tokens per block

N_CORES = 8


def build_program(use_hybrid_mm=True, t_total=T):
    """Build the single-core GRU program (SPMD: same program on all cores)."""
    import concourse.bacc as bacc
    import concourse.bass as bass
    import concourse.mybir as mybir
    import concourse.tile as tile
    from concourse.bass import ds
    from concourse.masks import make_identity

    nblk = t_total // TBLK
    assert nblk >= 2 and nblk % 2 == 0

    fp32 = mybir.dt.float32
    bf16 = mybir.dt.bfloat16
    AF = mybir.ActivationFunctionType
    OP = mybir.AluOpType

    # Bacc (not raw Bass): its compile() pass splits multi-sem waits into
    # EventSemaphore chains — walrus only accepts ONE sync wait per instruction
    nc = bacc.Bacc("TRN2")

    # ---- DRAM I/O ----
    x_ids = nc.dram_tensor("x_ids", [t_total, B, 1], mybir.dt.int32, kind="ExternalInput")
    emb_d = nc.dram_tensor("emb", [V, E], fp32, kind="ExternalInput")
    w_d = nc.dram_tensor("w", [E, G], fp32, kind="ExternalInput")
    ur_d = nc.dram_tensor("ur", [U, G], fp32, kind="ExternalInput")
    # b0pg[p, j] = b[0][128j+p] + (b[1][128j+p] if gate(j) in {z,r} else 0)
    b0pg_d = nc.dram_tensor("b0pg", [128, GT], fp32, kind="ExternalInput")
    # b1n[p, 8*?? k*B+b] = b[1][2048 + 128k + p], broadcast over b  (n-gate recurrent bias)
    b1n_d = nc.dram_tensor("b1n", [128, KT * B], fp32, kind="ExternalInput")
    h0t_d = nc.dram_tensor("h0t", [128, KT * B], fp32, kind="ExternalInput")
    # stacked identity for the hybrid reduce-transpose: [I32; I32; I32; I32]
    s4i_d = nc.dram_tensor("s4i", [128, 32], fp32, kind="ExternalInput")

    # y grouped by YB=4 steps: y_out[q, p, 256*s + 32*k + b] = h_{4q+s}[b, 128k+p]
    y_d = nc.dram_tensor("y_out", [t_total // 4, 128, 4 * KT * B], fp32,
                         kind="ExternalOutput")
    h_d = nc.dram_tensor("h_out", [128, KT * B], fp32, kind="ExternalOutput")

    with tile.TileContext(nc) as tc:
        with (
            tc.tile_pool(name="const", bufs=1) as cpool,
            tc.tile_pool(name="work", bufs=2) as wpool,
            tc.tile_pool(name="psum", bufs=1, space="PSUM") as ppool,
        ):
            # ---- persistent SBUF state ----
            ur_sb = cpool.tile([128, KT * G], bf16, name="ur_sb")    # 48KB/part
            w_sb = cpool.tile([128, ET * G], bf16, name="w_sb")      # 24KB/part
            xw_a = cpool.tile([128, TBLK * G // 4], bf16, name="xw_a")  # [128,12288] 24KB
            xw_b = cpool.tile([128, TBLK * G // 4], bf16, name="xw_b")
            h_sb = cpool.tile([128, KT * B], bf16, name="h_sb")
            b0pg = cpool.tile([128, GT], fp32, name="b0pg")
            b1n = cpool.tile([128, KT * B], fp32, name="b1n")
            s4i = cpool.tile([128, 32], bf16, name="s4i")
            ident = cpool.tile([128, 128], bf16, name="ident")

            make_identity(nc, ident[:, :])

            # ---- load + cast weights ----
            for k in range(KT):
                stg = wpool.tile([128, G], fp32, tag="wstage", bufs=2)
                nc.sync.dma_start(out=stg[:, :], in_=ur_d[128 * k:128 * (k + 1), :])
                nc.vector.tensor_copy(ur_sb[:, G * k:G * (k + 1)], stg[:, :])
            for e in range(ET):
                stg = wpool.tile([128, G], fp32, tag="wstage", bufs=2)
                nc.sync.dma_start(out=stg[:, :], in_=w_d[128 * e:128 * (e + 1), :])
                nc.vector.tensor_copy(w_sb[:, G * e:G * (e + 1)], stg[:, :])

            nc.sync.dma_start(out=b0pg[:, :], in_=b0pg_d[:, :])
            nc.sync.dma_start(out=b1n[:, :], in_=b1n_d[:, :])
            h0stg = wpool.tile([128, KT * B], fp32, tag="h0stg", bufs=1)
            nc.sync.dma_start(out=h0stg[:, :], in_=h0t_d[:, :])
            nc.vector.tensor_copy(h_sb[:, :], h0stg[:, :])
            s4stg = wpool.tile([128, 32], fp32, tag="s4stg", bufs=1)
            nc.sync.dma_start(out=s4stg[:, :], in_=s4i_d[:, :])
            nc.vector.tensor_copy(s4i[:, :], s4stg[:, :])

            # ------------------------------------------------------------------
            def phase1_block(t0, xw):
                """xW^T for steps [t0, t0+TBLK) -> xw tile.

                xw columns: [t_local(16) x g_tile? no]: col = 768*t_local + 32*j + b
                i.e. per-step contiguous [128, G/4] slabs, g-tile-major inside.
                t0 may be a loop register (additive offsets only).
                """
                # gather + transpose -> xeT [128, ET*TOKB]: col = TOKB*e + tok
                xet = wpool.tile([128, ET * TOKB], bf16, tag="xet", bufs=2)
                for i in range(TOKB // 128):  # 4 tok-tiles of 128 tokens
                    idx = wpool.tile([128, 1], mybir.dt.int32, tag="idx", bufs=2)
                    nc.gpsimd.dma_start(out=idx[:, :], in_=x_ids[ds(t0 + 4 * i, 4), :, :])
                    # bounce the indices through a Pool-compute copy so the
                    # indirect DMA's every dependency (index ready + WAR on xe,
                    # whose previous reader is also Pool compute) collapses to
                    # ONE Pool sem — walrus allows a single wait on DMAs
                    idx2 = wpool.tile([128, 1], mybir.dt.int32, tag="idx2", bufs=2)
                    nc.gpsimd.tensor_copy(idx2[:, :], idx[:, :])
                    xe = wpool.tile([128, E], fp32, tag="xe", bufs=2)
                    nc.gpsimd.indirect_dma_start(
                        out=xe[:, :],
                        out_offset=None,
                        in_=emb_d[:, :],
                        in_offset=bass.IndirectOffsetOnAxis(ap=idx2[:, :1], axis=0),
                    )
                    # cast on gpsimd: the gather's WAR on xe and the idx DMA
                    # ordering then stay on the Pool engine (program order, no
                    # sems) — walrus allows only ONE sem wait on dynamic DMAs
                    # and on LDWEIGHTS, so both the indirect DMA and the
                    # transpose must end up with a single foreign producer.
                    xeb = wpool.tile([128, E], bf16, tag="xeb", bufs=2)
                    nc.gpsimd.tensor_copy(xeb[:, :], xe[:, :])
                    for e in range(ET):
                        tp = ppool.tile([128, 128], bf16, tag="tp_ps", bufs=1)
                        nc.tensor.transpose(
                            out=tp[:, :], in_=xeb[:, 128 * e:128 * (e + 1)],
                            identity=ident[:, :],
                        )
                        nc.vector.tensor_copy(
                            xet[:, TOKB * e + 128 * i: TOKB * e + 128 * (i + 1)],
                            tp[:, :],
                        )
                # matmul: for each g-tile, xwT[g-tile] = sum_e W[e,g].T @ xeT[e]
                for j in range(GT):
                    ps = ppool.tile([128, TOKB], fp32, tag="p1_ps", bufs=2)
                    for e in range(ET):
                        nc.tensor.matmul(
                            ps[:, :],
                            lhsT=w_sb[:, G * e + 128 * j: G * e + 128 * (j + 1)],
                            rhs=xet[:, TOKB * e: TOKB * (e + 1)],
                            start=(e == 0), stop=(e == ET - 1),
                        )
                    # copy + bias + cast into xw at [t_local-major] layout:
                    # ps col = 32*t_local + b  ->  xw col = 768*t_local + 32*j + b
                    src = ps[:, :].rearrange("p (t b) -> p t b", b=B)
                    dst = xw[:, :].rearrange("p (t g b) -> p t g b", g=GT, b=B)[:, :, j, :]
                    eng = nc.vector if (j % 2 == 0) else nc.scalar
                    if eng is nc.vector:
                        nc.vector.tensor_scalar_add(dst, src, b0pg[:, j:j + 1])
                    else:
                        nc.scalar.activation(dst, src, AF.Identity, bias=b0pg[:, j:j + 1])

            # ------------------------------------------------------------------
            YB = 4  # steps per y-output DMA batch
            _yblk_cur = [None]

            def step(qbase, xw, s):
                """One recurrence step; consumes xw slab s (0..TBLK-1), updates
                h_sb, stages y into a YB-step SBUF block, DMAs it out every YB
                steps (fewer dynamic DRAM offsets -> fewer SP/ACT registers).
                qbase = (time of slab 0 of xw) // YB, possibly a register."""
                xw0 = (G // 4) * s  # 768*s: this step's slab in xw

                gps = ppool.tile([128, GT * B], fp32, tag="g_ps", bufs=1)
                if use_hybrid_mm:
                    # Two halves of the gate dim (PSUM budget): each half does
                    # stage1 (col-tiled partials) -> copy -> stage2 (transpose
                    # reduce via stacked identity).
                    H = G // 2  # 1536
                    for hh in range(2):
                        pps = ppool.tile([128, H], fp32, tag="p_ps", bufs=1)
                        for cg in range(4):
                            for kk in range(2):
                                k = 2 * cg + kk
                                for n in range(H // 512):  # 3
                                    c0 = G * k + H * hh + 512 * n
                                    nc.tensor.matmul(
                                        pps[32 * cg:32 * (cg + 1), 512 * n:512 * (n + 1)],
                                        lhsT=h_sb[:, 32 * k:32 * (k + 1)],
                                        rhs=ur_sb[:, c0:c0 + 512],
                                        start=(kk == 0), stop=(kk == 1),
                                        tile_position=(0, 32 * cg),
                                    )
                        # stage 1.5: PSUM -> SBUF bf16 (split DVE/ACT)
                        pb = wpool.tile([128, H], bf16, tag="pb", bufs=2)
                        nc.vector.tensor_copy(pb[:, 0:H // 2], pps[:, 0:H // 2])
                        nc.scalar.copy(pb[:, H // 2:H], pps[:, H // 2:H])
                        # stage 2: G[j-tile] = pb[:,j].T @ s4i
                        for j in range(GT // 2):
                            jj = (GT // 2) * hh + j
                            nc.tensor.matmul(
                                gps[:, 32 * jj:32 * (jj + 1)],
                                lhsT=pb[:, 128 * j:128 * (j + 1)],
                                rhs=s4i[:, :],
                                start=True, stop=True,
                            )
                else:
                    # direct: G[j-tile] = sum_k U_r[k,j].T @ hT[k]
                    for j in range(GT):
                        for k in range(KT):
                            nc.tensor.matmul(
                                gps[:, 32 * j:32 * (j + 1)],
                                lhsT=ur_sb[:, G * k + 128 * j: G * k + 128 * (j + 1)],
                                rhs=h_sb[:, 32 * k:32 * (k + 1)],
                                start=(k == 0), stop=(k == KT - 1),
                            )

                # ---- gates ----
                # layout: cols [0:256) = z, [256:512) = r, [512:768) = n
                C = KT * B  # 256
                zr = wpool.tile([128, 2 * C], bf16, tag="zr", bufs=2)
                nc.vector.tensor_tensor(
                    out=zr[:, :], in0=gps[:, 0:2 * C],
                    in1=xw[:, xw0:xw0 + 2 * C], op=OP.add,
                )
                zt = wpool.tile([128, C], bf16, tag="zt", bufs=2)
                rt = wpool.tile([128, C], bf16, tag="rt", bufs=2)
                nc.scalar.activation(zt[:, :], zr[:, 0:C], AF.Sigmoid)
                nc.scalar.activation(rt[:, :], zr[:, C:2 * C], AF.Sigmoid)
                # hnb = hn + b1n
                hnb = wpool.tile([128, C], bf16, tag="hnb", bufs=2)
                nc.vector.tensor_tensor(
                    out=hnb[:, :], in0=gps[:, 2 * C:3 * C], in1=b1n[:, :], op=OP.add,
                )
                # t2 = r*hnb ; t3 = t2 + xw_n ; n = tanh(t3)
                t2 = wpool.tile([128, C], bf16, tag="t2", bufs=2)
                nc.vector.tensor_mul(t2[:, :], rt[:, :], hnb[:, :])
                t3 = wpool.tile([128, C], bf16, tag="t3", bufs=2)
                nc.vector.tensor_tensor(
                    out=t3[:, :], in0=t2[:, :], in1=xw[:, xw0 + 2 * C:xw0 + 3 * C], op=OP.add,
                )
                nt = wpool.tile([128, C], bf16, tag="nt", bufs=2)
                nc.scalar.activation(nt[:, :], t3[:, :], AF.Tanh)
                # h_new = n + z*(h - n)
                dt_ = wpool.tile([128, C], bf16, tag="dt", bufs=2)
                nc.vector.tensor_sub(dt_[:, :], h_sb[:, :], nt[:, :])
                hf = wpool.tile([128, C], bf16, tag="hf", bufs=2)
                nc.vector.tensor_mul(hf[:, :], zt[:, :], dt_[:, :])
                # h_new written straight into the y staging block (one tile
                # per YB-step group; same buffer must back all YB slices)
                if s % YB == 0:
                    _yblk_cur[0] = wpool.tile(
                        [128, YB * C], fp32, tag="yblk", bufs=2, name="yblk"
                    )
                yblk = _yblk_cur[0]
                hf2 = yblk[:, (s % YB) * C:(s % YB + 1) * C]
                nc.vector.tensor_tensor(out=hf2, in0=hf[:, :], in1=nt[:, :], op=OP.add)
                # update state + emit y every YB steps
                nc.vector.tensor_copy(h_sb[:, :], hf2)
                if s % YB == YB - 1:
                    dma_eng = [nc.sync, nc.scalar][(s // YB) % 2]
                    dma_eng.dma_start(
                        out=y_d[ds(qbase + s // YB, 1), :, :],
                        in_=yblk[:, :],
                    )
                return hf2

            # ------------------------------------------------------------------
            # prologue: blocks 0 and 1
            phase1_block(0, xw_a)
            phase1_block(TBLK, xw_b)

            last_hf = None
            QB = TBLK // YB  # 4 y-groups per block
            if nblk > 2:
                # loop variable counts y-groups (YB steps each) so that both
                # q-unit (y) and t-unit (x_ids, via *YB) offsets are affine
                with tc.For_i(0, (nblk - 2) * QB, 2 * QB) as q0:
                    for s in range(TBLK):
                        step(q0, xw_a, s)
                    phase1_block(q0 * YB + 2 * TBLK, xw_a)
                    for s in range(TBLK):
                        step(q0 + QB, xw_b, s)
                    phase1_block(q0 * YB + 3 * TBLK, xw_b)
            # epilogue: last two blocks
            te = (nblk - 2) * TBLK
            for s in range(TBLK):
                step(te // YB, xw_a, s)
            for s in range(TBLK):
                last_hf = step(te // YB + QB, xw_b, s)

            nc.sync.dma_start(out=h_d[:, :], in_=last_hf[:, :])

    nc.finalize()
    return nc


# ----------------------------------------------------------------------------
# host-side packing / unpacking
# ----------------------------------------------------------------------------

def _pack_inputs(x_tb, emb, w, ur, b, h0):
    """Per-core input map for one direction. x_tb: [T, B] int32 (already
    time-ordered for this direction)."""
    b = np.asarray(b, np.float32)
    b0, b1 = b[0], b[1]
    # b0pg[p, j] = b0[128j+p] + (b1[128j+p] if j < 16 else 0)   (z,r tiles)
    badd = b0 + np.where(np.arange(G) < 2 * U, b1, 0.0)
    b0pg = badd.reshape(GT, 128).T.copy()                       # [128, GT]
    # b1n[p, 32k+b] = b1[2048 + 128k + p]
    b1n = np.broadcast_to(
        b1[2 * U:].reshape(KT, 128).T[:, :, None], (128, KT, B)
    ).reshape(128, KT * B).copy()
    # h0t[p, 32k+b] = h0[b, 128k+p]
    h0t = np.ascontiguousarray(
        np.asarray(h0, np.float32).reshape(B, KT, 128).transpose(2, 1, 0)
    ).reshape(128, KT * B)
    s4i = np.tile(np.eye(32, dtype=np.float32), (4, 1))         # [128, 32]
    return {
        "x_ids": np.ascontiguousarray(x_tb, np.int32).reshape(T, B, 1),
        "emb": np.asarray(emb, np.float32),
        "w": np.asarray(w, np.float32),
        "ur": np.asarray(ur, np.float32),
        "b0pg": np.ascontiguousarray(b0pg),
        "b1n": np.ascontiguousarray(b1n),
        "h0t": h0t,
        "s4i": s4i,
    }


def _unpack_y(y_raw):
    """y_raw [T//4, 128, 4*KT*B] -> y [B, T, U];
    y_raw[q, p, 256*s + 32*k + b] = y[b, 4q+s, 128k+p]."""
    return np.ascontiguousarray(
        y_raw.reshape(T // 4, 128, 4, KT, B).transpose(4, 0, 2, 3, 1)
    ).reshape(B, T, U)


def _unpack_h(h_raw):
    """h_raw [128, KT*B] -> h [B, U]."""
    return np.ascontiguousarray(
        h_raw.reshape(128, KT, B).transpose(2, 1, 0)
    ).reshape(B, U)


_CACHED = {}


def _get_program():
    if "nc" not in _CACHED:
        _CACHED["nc"] = build_program()
    return _CACHED["nc"]


def kernel(x, emb, W_f, U_f, b_f, W_b, U_b, b_b, h0_f, h0_b, _trace=False):
    from concourse.bass_utils import run_bass_kernel_spmd

    x = np.asarray(x)
    x_f = x.T                          # [T, B] forward time order
    x_b = x.T[::-1]                    # reversed time order

    in_f = _pack_inputs(x_f, emb, W_f, U_f, b_f, h0_f)
    in_b = _pack_inputs(x_b, emb, W_b, U_b, b_b, h0_b)

    in_maps = []
    for c in range(N_CORES):
        in_maps.append(in_f if c % 2 == 0 else in_b)

    nc = _get_program()
    res = run_bass_kernel_spmd(
        nc, in_maps, core_ids=list(range(N_CORES)), trace=_trace,
    )
    rf, rb = res.results[0], res.results[1]

    y_f = _unpack_y(rf["y_out"])
    y_b = _unpack_y(rb["y_out"])[:, ::-1]     # un-reverse time
    h_f = _unpack_h(rf["h_out"])
    h_b = _unpack_h(rb["h_out"])
    y = np.concatenate([y_f, y_b], axis=-1)   # [B, T, 2U]
    if _trace:
        kernel.last_exec_ns = res.exec_time_ns
        kernel.last_results = res
    return (np.ascontiguousarray(y, np.float32), h_f.astype(np.float32),
            h_b.astype(np.float32))
